# revision 64
# baseline (speedup 1.0000x reference)
"""Multi-head attention (B=2, S=2048, D=1024, H=16) on 8 TRN2 NeuronCores.

Sharding: (batch, head-group) - core c handles batch c//4 and heads
[4*(c%4), 4*(c%4)+4). Each core projects its batch's tokens onto its 4 heads'
column-shards of Wq/Wk/Wv, runs attention for those heads, and multiplies by
its row-shard of Wo, producing a partial [S, D] output. The host sums the 4
partials per batch and adds bo. No FLOP duplication across cores.

Device design notes (v2, e-stationary PV):
  - Q/K are projected feature-major (qT/kT [dims, tokens] f32) so QK^T streams
    queries: scores^T [keys, queries] per 128-key block, exp'd on ACT into
    bf16 e tiles [128 keys, 1024 queries].
  - PV uses e as the STATIONARY operand: ctx[q, d] = e_blk^T @ v_blk with
    v [128 keys, 64 dims] as the moving operand (N=64), accumulated over key
    blocks in PSUM. Output lands queries-on-partitions, so the softmax
    denominator divide is a per-partition tensor_scalar multiply (no
    partition broadcasts). Denominators come from parallel N=1 matmuls
    (e_blk^T @ ones) accumulated in a dedicated PSUM bank.
  - V is projected token-major (x-chunk stationary, Wv moving, N=256), which
    directly yields v [tokens, dims] - no V transposes.
  - Normalized ctx pairs are PE-transposed ([q, dims] -> [dims, q]) into the
    packed ctx_t layout for the row-sharded Wo matmul (bf16).
  - The j-loop is split in two rounds (key halves) so attention overlaps the
    input-DMA ramp; round-1 ctx partials are evicted to SBUF and re-added
    during round 2. Denominators accumulate across both rounds in PSUM.
  - PSUM budget (8 banks): qk 2x[128,1024] (4) + ctx 2x[128,512] (2) +
    denominators (1) + scratch for proj/wo/transpose groups (1).  The ramp
    projections trickle per-DMA-chunk into the (still unused) qk psum slots;
    late projections run group-at-a-time from persistent stage tiles through
    the scratch bank so no psum slot is ever held across interleaved work.
  - Eviction work is spread: ACT (ramp proj bias adds), DVE (late proj bias,
    V bias adds, R1 evict, R2 add, reciprocal, ctx_t + Wo psum evictions),
    Pool/gpsimd (normalize multiplies - SBUF-only, since gpsimd has no PSUM
    port).
"""

import numpy as np

S = 2048          # sequence length
D = 1024          # model dim
HPC = 4           # heads per core
DK = 64           # head dim
M = HPC * DK      # per-core projection width = 256
NC = 8            # cores
IH = S // 2       # query half width (free dim of qk/exp tiles)
NQB = IH // 128   # 8 query blocks per half
NDC = D // 128    # 8 contraction chunks

IN_BF16 = True    # stream q/k/v inputs (and Wq/Wk) as bf16

_cached = {}


def _build(debug=False):
    import concourse.bass as bass
    import concourse.bacc as bacc
    import concourse.tile as tile
    import concourse.mybir as mybir
    from contextlib import ExitStack
    from collections import deque

    f32 = mybir.dt.float32
    f32r = mybir.dt.float32r
    bf16 = mybir.dt.bfloat16
    f16 = mybir.dt.float16
    AF = mybir.ActivationFunctionType

    xdt = bf16 if IN_BF16 else f32

    def r(ap):
        # moving/stationary f32 operands go through the PE at full rate as f32r
        return ap.bitcast(f32r) if ap.dtype == f32 else ap

    nc = bacc.Bacc(
        "TRN2",
        target_bir_lowering=False,
        debug=False,
        enable_asserts=False,
        num_devices=NC,
    )

    xqT_d = nc.dram_tensor("xqT", [D, S], xdt, kind="ExternalInput").ap()
    xkT_d = nc.dram_tensor("xkT", [D, S], xdt, kind="ExternalInput").ap()
    xvT_d = nc.dram_tensor("xvT", [D, S], bf16, kind="ExternalInput").ap()
    wq_d = nc.dram_tensor("wq", [D, M], xdt, kind="ExternalInput").ap()
    wk_d = nc.dram_tensor("wk", [D, M], xdt, kind="ExternalInput").ap()
    wv_d = nc.dram_tensor("wv", [D, M], bf16, kind="ExternalInput").ap()
    wo_d = nc.dram_tensor("wo", [M, D], bf16, kind="ExternalInput").ap()
    bq_d = nc.dram_tensor("bq", [M], f32, kind="ExternalInput").ap()
    bk_d = nc.dram_tensor("bk", [M], f32, kind="ExternalInput").ap()
    bvb_d = nc.dram_tensor("bvb", [128, M], f32, kind="ExternalInput").ap()
    ident_d = nc.dram_tensor("ident", [128, 128], bf16, kind="ExternalInput").ap()
    out_d = nc.dram_tensor("out", [S, D], f16, kind="ExternalOutput").ap()

    with tile.TileContext(nc) as tc:
        with ExitStack() as st:
            # ---- SBUF pools ----
            pw = st.enter_context(tc.tile_pool(name="pw", bufs=1))
            pqk = st.enter_context(tc.tile_pool(name="pqk", bufs=1))
            pvs = st.enter_context(tc.tile_pool(name="pvs", bufs=1))
            pxv = st.enter_context(tc.tile_pool(name="pxv", bufs=1))
            pstg = st.enter_context(tc.tile_pool(name="pstg", bufs=1))
            pct = st.enter_context(tc.tile_pool(name="pct", bufs=1))
            xt = st.enter_context(tc.tile_pool(name="xt", bufs=8))
            ep = st.enter_context(tc.tile_pool(name="ep", bufs=18))
            cpp = st.enter_context(tc.tile_pool(name="cpp", bufs=5))
            tmpp = st.enter_context(tc.tile_pool(name="tmpp", bufs=2))
            invp = st.enter_context(tc.tile_pool(name="invp", bufs=2))
            ostp = st.enter_context(tc.tile_pool(name="ostp", bufs=6))
            # ---- PSUM pools (8 banks total) ----
            qp = st.enter_context(tc.tile_pool(name="qp", bufs=2, space="PSUM"))
            cxp = st.enter_context(tc.tile_pool(name="cxp", bufs=2, space="PSUM"))
            dnp = st.enter_context(tc.tile_pool(name="dnp", bufs=1, space="PSUM"))
            pps = st.enter_context(tc.tile_pool(name="pps", bufs=1, space="PSUM"))

            # ---- persistent SBUF tiles ----
            qT = [[pqk.tile([128, IH], f32r, tag=f"qT{m}{s}", name=f"qT{m}{s}")
                   for s in range(2)] for m in range(2)]
            kT = [[pqk.tile([128, IH], f32r, tag=f"kT{m}{s}", name=f"kT{m}{s}")
                   for s in range(2)] for m in range(2)]
            v_sb = [[pvs.tile([128, 8, DK], bf16, tag=f"v{h}{s}", name=f"v{h}{s}")
                     for s in range(2)] for h in range(HPC)]
            ctx_t = [pct.tile([128, 2, IH], bf16, tag=f"ctxt{i}", name=f"ctxt{i}")
                     for i in range(2)]
            cpair = [[pct.tile([128, NQB, 128], bf16, tag=f"cp{i}{m}",
                               name=f"cp{i}{m}") for m in range(2)]
                     for i in range(2)]

            wq_sb = pw.tile([128, NDC, M], xdt, tag="wq")
            wk_sb = pw.tile([128, NDC, M], xdt, tag="wk")
            wv_sb = pw.tile([128, NDC, M], bf16, tag="wv")
            wo_sb = pw.tile([128, 2, D], bf16, tag="wo")
            bq_sb = pw.tile([128, 2], f32, tag="bq")
            bk_sb = pw.tile([128, 2], f32, tag="bk")
            bvb_sb = pw.tile([128, M], f32, tag="bvb")
            ident = pw.tile([128, 128], bf16, tag="ident")
            ones = pw.tile([128, 1], bf16, tag="ones")

            # denominator accumulator: col = ih*32 + h*8 + qb
            dn = dnp.tile([128, 64], f32, tag="dn", name="dn")

            w_r = lambda ap: ap.rearrange("(n p) m -> p n m", p=128)

            nc.vector.memset(ones, 1.0)

            # ---------------- emission helpers ----------------
            fillers = deque()

            def pull(n=1):
                for _ in range(n):
                    while fillers:
                        try:
                            next(fillers[0])
                            break
                        except StopIteration:
                            fillers.popleft()
                    else:
                        return

            qchunks = []
            kchunks = []

            def ramp_qk_proj(tens, mcs):
                """Ramp projection of q/k token-half 0: x chunks trickle from
                DMA straight into accumulating matmuls hosted in the (still
                free) qk psum slots.  Runs before any attention emission.
                Only head-pairs in `mcs` are projected; for q, mc1 is
                deferred to a filler (the first attention heads are mc0)."""
                xdram = xqT_d if tens == "q" else xkT_d
                w_sb = wq_sb if tens == "q" else wk_sb
                b_sb = bq_sb if tens == "q" else bk_sb
                dst = qT if tens == "q" else kT
                ps = {mc: qp.tile([128, IH], f32, tag="qk", name=f"pj{tens}{mc}")
                      for mc in mcs}
                for dc in range(NDC):
                    xc = xt.tile([128, IH], xdt, tag="x", name="x")
                    nc.sync.dma_start(out=xc, in_=xdram[dc * 128:(dc + 1) * 128, 0:IH])
                    (qchunks if tens == "q" else kchunks).append(xc)
                    for mc in mcs:
                        for sc in range(2):
                            nc.tensor.matmul(
                                ps[mc][:, sc * 512:(sc + 1) * 512],
                                lhsT=r(w_sb[:, dc, mc * 128:(mc + 1) * 128]),
                                rhs=r(xc[:, sc * 512:(sc + 1) * 512]),
                                start=(dc == 0),
                                stop=(dc == NDC - 1),
                            )
                for mc in mcs:
                    # sc0 on ACT / sc1 on DVE: the two halves evict in
                    # parallel so first-exp follows the last matmul quickly
                    nc.scalar.add(
                        out=dst[mc][0][:, 0:512],
                        in_=ps[mc][:, 0:512],
                        add=b_sb[:, mc:mc + 1])
                    nc.vector.tensor_scalar_add(
                        out=dst[mc][0][:, 512:1024],
                        in0=ps[mc][:, 512:1024],
                        scalar1=b_sb[:, mc:mc + 1])

            def emit_late_mc1(tens):
                """Deferred mc1 projection of q/k half-0 from the saved ramp
                chunks, one group at a time through the scratch bank."""
                w_sb = wq_sb if tens == "q" else wk_sb
                b_sb = bq_sb if tens == "q" else bk_sb
                dst = (qT if tens == "q" else kT)[1][0]
                chunks = qchunks if tens == "q" else kchunks
                for sc in range(2):
                    ps = pps.tile([128, 512], f32, tag="ps", name=f"{tens}mc1")
                    for dc in range(NDC):
                        nc.tensor.matmul(
                            ps,
                            lhsT=r(w_sb[:, dc, 128:256]),
                            rhs=r(chunks[dc][:, sc * 512:(sc + 1) * 512]),
                            start=(dc == 0),
                            stop=(dc == NDC - 1),
                        )
                        if dc == 3:
                            yield
                    nc.vector.tensor_scalar_add(
                        out=dst[:, sc * 512:(sc + 1) * 512],
                        in0=ps, scalar1=b_sb[:, 1:2])
                    yield

            stg_tiles = {}

            def emit_stage_dma(tens):
                """DMA the token-half-1 x chunks of q/k into a persistent
                stage tile (SP queue only - no engine work)."""
                xdram = xqT_d if tens == "q" else xkT_d
                stg = pstg.tile([128, NDC, IH], xdt, tag="stg", name=f"stg{tens}")
                for dc in range(NDC):
                    nc.sync.dma_start(
                        out=stg[:, dc, :],
                        in_=xdram[dc * 128:(dc + 1) * 128, IH:S])
                    yield
                stg_tiles[tens] = stg

            def emit_late_proj(tens):
                """Token-half-1 projection of q/k from the stage tile,
                one (mc, sc) accumulation group at a time through the
                scratch psum bank."""
                w_sb = wq_sb if tens == "q" else wk_sb
                b_sb = bq_sb if tens == "q" else bk_sb
                dst = qT if tens == "q" else kT
                stg = stg_tiles[tens]
                for mc in range(2):
                    for sc in range(2):
                        ps = pps.tile([128, 512], f32, tag="ps", name=f"lp{tens}")
                        for dc in range(NDC):
                            nc.tensor.matmul(
                                ps,
                                lhsT=r(w_sb[:, dc, mc * 128:(mc + 1) * 128]),
                                rhs=r(stg[:, dc, sc * 512:(sc + 1) * 512]),
                                start=(dc == 0),
                                stop=(dc == NDC - 1),
                            )
                            if dc % 2 == 1:
                                yield
                        nc.vector.tensor_scalar_add(
                            out=dst[mc][1][:, sc * 512:(sc + 1) * 512],
                            in0=ps, scalar1=b_sb[:, mc:mc + 1])
                        yield

            xv_tiles = {}

            def emit_xv_dma(sh):
                xv = pxv.tile([128, NDC, IH], bf16, tag="xv", name=f"xv{sh}")
                for dc in range(NDC):
                    nc.sync.dma_start(
                        out=xv[:, dc, :],
                        in_=xvT_d[dc * 128:(dc + 1) * 128, sh * IH:(sh + 1) * IH])
                    yield
                xv_tiles[sh] = xv

            def emit_v_proj(sh):
                """Token-major V projection: two token-blocks per pps tile."""
                xv = xv_tiles[sh]
                for tbp in range(4):
                    ps = pps.tile([128, 512], f32, tag="ps", name="vps")
                    for dc in range(NDC):
                        for j in range(2):
                            tb = tbp * 2 + j
                            # the two token-blocks share one psum bank:
                            # single start (j0/dc0) and stop (j1/dc7)
                            nc.tensor.matmul(
                                ps[:, j * M:(j + 1) * M],
                                lhsT=xv[:, dc, tb * 128:(tb + 1) * 128],
                                rhs=wv_sb[:, dc, :],
                                start=(dc == 0 and j == 0),
                                stop=(dc == NDC - 1 and j == 1),
                            )
                        if dc % 2 == 1:
                            yield
                    for j in range(2):
                        tb = tbp * 2 + j
                        for h in range(HPC):
                            nc.vector.tensor_add(
                                v_sb[h][sh][:, tb, :],
                                ps[:, j * M + h * DK:j * M + (h + 1) * DK],
                                bvb_sb[:, h * DK:(h + 1) * DK])
                    yield

            def emit_wo_dma():
                nc.sync.dma_start(out=wo_sb, in_=w_r(wo_d))
                yield

            def emit_wo(ih, icb, tail=False):
                """One token-block of the output projection (both D halves).

                In tail mode the two psum groups alternate between a qk slot
                (free by then) and the scratch bank, and the evictions
                alternate ACT/DVE, so the final token-blocks pipeline instead
                of serializing on one bank + one engine."""
                ic = ih * NQB + icb
                for nh in range(2):
                    if tail and nh == 0:
                        ps = qp.tile([128, 512], f32, tag="qk", name="wops")
                    else:
                        ps = pps.tile([128, 512], f32, tag="ps", name="wops")
                    for g in range(2):
                        nc.tensor.matmul(
                            ps,
                            lhsT=ctx_t[ih][:, g, icb * 128:(icb + 1) * 128],
                            rhs=wo_sb[:, g, nh * 512:(nh + 1) * 512],
                            start=(g == 0),
                            stop=(g == 1),
                        )
                    st_ = ostp.tile([128, 512], f16, tag="ost", name="st")
                    if tail and nh == 1:
                        nc.scalar.activation(out=st_, in_=ps, func=AF.Copy)
                    else:
                        nc.vector.tensor_copy(out=st_, in_=ps)
                    # out-DMAs ride the SP queue: issuing from the ACT queue
                    # would stall the exp decode stream ~650ns per DMA
                    nc.sync.dma_start(
                        out=out_d[ic * 128:(ic + 1) * 128, nh * 512:(nh + 1) * 512],
                        in_=st_)
                    yield

            cxs = {}      # (ih, h) -> live ctx psum tile
            cps = {}      # (ih, h) -> R1 partial in SBUF

            def emit_qk_exp(ih, h, kb):
                sh, kbl = divmod(kb, 8)
                mc, off = divmod(h, 2)
                off *= DK
                qk = qp.tile([128, IH], f32, tag="qk", name="qk")
                for ha in range(2):
                    nc.tensor.matmul(
                        qk[:, ha * 512:(ha + 1) * 512],
                        lhsT=r(kT[mc][sh][off:off + DK, kbl * 128:(kbl + 1) * 128]),
                        rhs=r(qT[mc][ih][off:off + DK, ha * 512:(ha + 1) * 512]),
                        start=True, stop=True,
                    )
                e = ep.tile([128, IH], bf16, tag="e", name="e")
                nc.scalar.activation(out=e, in_=qk, func=AF.Exp, scale=1.0 / 8.0)
                return e

            dn_state = {"started": False}

            def emit_pv(ih, h, kb, e):
                # PSUM group semantics: a start=True matmul zeroes the whole
                # 2KB bank, so exactly ONE start (first matmul into the bank)
                # and ONE stop (last matmul) per bank-lifetime; all sibling
                # accumulators in the bank ride the same group and simply
                # accumulate onto the zeroed region.
                sh = kb // 8
                if kb % 8 == 0:
                    cxs[(ih, h)] = cxp.tile([128, 512], f32, tag="ctx",
                                            name=f"cx{ih}{h}{kb}")
                cx = cxs[(ih, h)]
                dbase = ih * 32 + h * 8
                for qb in range(NQB):
                    lhs = e[:, qb * 128:(qb + 1) * 128]
                    nc.tensor.matmul(
                        cx[:, qb * DK:(qb + 1) * DK],
                        lhsT=lhs,
                        rhs=v_sb[h][sh][:, kb % 8, :],
                        start=(kb % 8 == 0 and qb == 0),
                        stop=(kb % 8 == 7 and qb == NQB - 1),
                    )
                    dn_start = not dn_state["started"]
                    dn_state["started"] = True
                    # the bank-wide group stays open across the whole kernel
                    # (per-slot reads are ordered by data deps, so skip the
                    # interp's group-state checks for these)
                    nc.tensor.matmul(
                        dn[:, dbase + qb:dbase + qb + 1],
                        lhsT=lhs,
                        rhs=ones,
                        start=dn_start,
                        stop=(ih == 1 and h == 3 and kb == 15 and qb == NQB - 1),
                        skip_group_check=True,
                    )

            def emit_evict_r1(ih, h):
                cx = cxs.pop((ih, h))
                cp = cpp.tile([128, 512], f32, tag="cp", name=f"cp{ih}{h}")
                nc.vector.tensor_copy(out=cp, in_=cx)
                cps[(ih, h)] = cp

            def emit_norm(ih, h):
                cx = cxs.pop((ih, h))
                cp = cps.pop((ih, h))
                mc, off = divmod(h, 2)
                off *= DK
                dbase = ih * 32 + h * 8
                inv = invp.tile([128, NQB], f32, tag="inv", name="inv")
                nc.vector.reciprocal(out=inv, in_=dn[:, dbase:dbase + NQB])
                tm = tmpp.tile([128, 512], f32, tag="tmp", name="tm")
                nc.vector.tensor_add(tm, cx, cp)
                for qb in range(NQB):
                    nc.gpsimd.tensor_scalar_mul(
                        out=cpair[ih][mc][:, qb, off:off + DK],
                        in0=tm[:, qb * DK:(qb + 1) * DK],
                        scalar1=inv[:, qb:qb + 1])

            def emit_tp(ih, mc, qb):
                # transposes borrow a ctx psum slot (never the scratch bank,
                # which may be mid-accumulation inside a filler generator)
                tp = cxp.tile([128, 128], bf16, tag="ctx", name="tp")
                nc.tensor.transpose(tp, in_=cpair[ih][mc][:, qb, :], identity=ident)
                nc.vector.tensor_copy(
                    out=ctx_t[ih][:, mc, qb * 128:(qb + 1) * 128], in_=tp)

            def emit_tp_half(ih, mc, qb, half, eng="dve"):
                # one head's 64-dim half of a pair transpose: [128 q, 64 d]
                # -> psum partitions half*64..half*64+64, then a partition-
                # sliced copy into ctx_t.  Lets the tail pair's even head
                # transpose early, leaving only the odd half on the tail.
                tp = cxp.tile([128, 128], bf16, tag="ctx", name="tph")
                nc.tensor.transpose(
                    tp[half * DK:(half + 1) * DK, :],
                    in_=cpair[ih][mc][:, qb, half * DK:(half + 1) * DK],
                    identity=ident)
                dst = ctx_t[ih][half * DK:(half + 1) * DK, mc,
                               qb * 128:(qb + 1) * 128]
                if eng == "act":
                    nc.scalar.activation(
                        out=dst, in_=tp[half * DK:(half + 1) * DK, :],
                        func=AF.Copy)
                else:
                    nc.vector.tensor_copy(
                        out=dst, in_=tp[half * DK:(half + 1) * DK, :])

            # ---------------- global schedule ----------------
            # PE p-state: the cost model resets the tensor clock to 0.65/1.2
            # GHz after any dependency wait and needs ~3us of continuous
            # execution to reach 2.4 GHz.  An initial burst of junk matmuls
            # on a zeroed tile warms the engine before the chunk-paced
            # projections; from then on 4 matmuls per 728ns chunk period
            # keep the engine saturated (never waiting, never resetting).
            junk = pw.tile([128, 512], bf16, tag="junk")
            nc.vector.memset(junk, 0.0)
            jps = pps.tile([128, 512], f32, tag="ps", name="jps")
            for _ in range(8):
                nc.tensor.matmul(jps, lhsT=junk[:, 0:128], rhs=junk,
                                 start=True, stop=True)
            nc.sync.dma_start(out=wk_sb, in_=w_r(wk_d))
            nc.sync.dma_start(out=bk_sb, in_=bk_d.rearrange("(n p) -> p n", p=128))
            ramp_qk_proj("k", (0, 1))
            nc.sync.dma_start(out=wq_sb, in_=w_r(wq_d))
            nc.sync.dma_start(out=bq_sb, in_=bq_d.rearrange("(n p) -> p n", p=128))
            ramp_qk_proj("q", (0, 1))
            nc.sync.dma_start(out=wv_sb, in_=w_r(wv_d))
            nc.sync.dma_start(out=bvb_sb, in_=bvb_d)
            nc.sync.dma_start(out=ident, in_=ident_d)
            for _ in emit_xv_dma(0):
                pass

            # fillers consumed inside attention (FIFO order matters: each
            # generator's data deps are satisfied by the time it is pulled)
            fillers.append(emit_stage_dma("k"))
            fillers.append(emit_v_proj(0))
            fillers.append(emit_late_proj("k"))
            fillers.append(emit_xv_dma(1))
            fillers.append(emit_v_proj(1))
            fillers.append(emit_wo_dma())
            fillers.append(emit_stage_dma("q"))
            fillers.append(emit_late_proj("q"))

            es = {}
            # ---- R1(ih0): heads 0,1 qk/exp only (V still streaming);
            # aggressive pulls here drain the deferred mc1 projections
            # before heads 2,3 need them ----
            for h in (0, 1):
                for kb in range(8):
                    es[(h, kb)] = emit_qk_exp(0, h, kb)
                    pull(3)
            # ---- heads 2,3 qk/exp, flushing heads 0,1 PV with a lag ----
            for h in (2, 3):
                for kb in range(8):
                    es[(h, kb)] = emit_qk_exp(0, h, kb)
                    emit_pv(0, h - 2, kb, es.pop((h - 2, kb)))
                    pull(2)
                emit_evict_r1(0, h - 2)

            # ---- Lag-2 software pipeline for the remaining three rounds:
            # the qk/exp of iterations i+1 AND i+2 are emitted before the pv
            # of iteration i, so each qk completes well inside the previous
            # exp's window and ACT never waits on the in-order PE queue.
            # drain(it) emits the pv (+ any round-boundary work) of `it`.
            def drain(it):
                ih, ph, pkb, pe = it
                extra = False
                if ih == 0 and pkb >= 8 and ph < 2:
                    # leftover R1 pv of heads 2,3 rides on heads 0,1 of R2
                    emit_pv(0, ph + 2, pkb - 8, es.pop((ph + 2, pkb - 8)))
                    extra = True
                    if pkb == 15:
                        emit_evict_r1(0, ph + 2)
                emit_pv(ih, ph, pkb, pe)
                if pkb == 7 and not (ih == 0 and ph >= 2):
                    emit_evict_r1(ih, ph)
                boundary = False
                if pkb == 15 and not (ih == 1 and ph == 3):
                    emit_norm(ih, ph)
                    if ph % 2 == 1:
                        for qb in range(NQB):
                            emit_tp(ih, ph // 2, qb)
                        boundary = True
                if not boundary:
                    pull(2 if (ih == 0 and ph == 0) else 1)

            seq = ([(0, h, kb) for h in range(HPC) for kb in range(8, 16)]
                   + [(1, h, kb) for h in range(HPC) for kb in range(8)]
                   + [(1, h, kb) for h in range(HPC) for kb in range(8, 16)])
            wo0 = False
            pend = deque()
            for ih, h, kb in seq:
                if not wo0 and (ih, h, kb) == (1, 0, 2):
                    for icb in range(NQB):
                        fillers.append(emit_wo(0, icb))
                    wo0 = True
                e = emit_qk_exp(ih, h, kb)
                pend.append((ih, h, kb, e))
                if len(pend) > 2:
                    drain(pend.popleft())
            while pend:
                drain(pend.popleft())

            # ---- tail: normalize the last head per query block and
            # immediately transpose + project + store that block.  Everything
            # is per-qb so the 6-stage chain (DVE add -> Pool mul -> PE
            # transpose -> DVE copy -> PE wo -> ACT/DVE evict -> DMA)
            # pipelines across engines; wo psums rotate over the qk/ctx/
            # scratch banks (all free by now) and evictions alternate
            # ACT/DVE so no single bank or engine serializes the tail. ----
            inv = invp.tile([128, NQB], f32, tag="inv", name="inv")
            nc.vector.reciprocal(out=inv, in_=dn[:, 56:64])
            cx = cxs.pop((1, 3))
            cp = cps.pop((1, 3))
            # Pool pre-scales the R1 partial by 1/denom so one fused DVE
            # scalar_tensor_tensor per block does (psum*inv + partial*inv)
            tm = tmpp.tile([128, 512], f32, tag="tmp", name="tm")

            def tail_psum(u):
                if u % 3 == 0:
                    return qp.tile([128, 512], f32, tag="qk", name="wops")
                if u % 3 == 1:
                    return cxp.tile([128, 512], f32, tag="ctx", name="wops")
                return pps.tile([128, 512], f32, tag="ps", name="wops")

            mult_op = mybir.AluOpType.mult
            add_op = mybir.AluOpType.add
            # phase A: the whole normalize as an uninterrupted DVE/Pool burst
            # (no wo-evictions queued behind it to couple the stages)
            for qb in range(NQB):
                nc.gpsimd.tensor_scalar_mul(
                    out=tm[:, qb * DK:(qb + 1) * DK],
                    in0=cp[:, qb * DK:(qb + 1) * DK],
                    scalar1=inv[:, qb:qb + 1])
                nc.vector.scalar_tensor_tensor(
                    out=cpair[1][1][:, qb, DK:2 * DK],
                    in0=cx[:, qb * DK:(qb + 1) * DK],
                    scalar=inv[:, qb:qb + 1],
                    in1=tm[:, qb * DK:(qb + 1) * DK],
                    op0=mult_op, op1=add_op)
            # phase B: per-qb transpose + wo, psums rotating over all five
            # free psum slots, evictions alternating ACT/DVE
            nslot = 0

            def tail_psum(shape, dtype):
                nonlocal nslot
                nslot += 1
                if nslot % 5 in (0, 1):
                    return qp.tile(shape, dtype, tag="qk", name="wops")
                if nslot % 5 in (2, 3):
                    return cxp.tile(shape, dtype, tag="ctx", name="wops")
                return pps.tile(shape, dtype, tag="ps", name="wops")

            def tail_tp(qb):
                tp = tail_psum([128, 128], bf16)
                nc.tensor.transpose(tp, in_=cpair[1][1][:, qb, :], identity=ident)
                nc.scalar.activation(
                    out=ctx_t[1][:, 1, qb * 128:(qb + 1) * 128], in_=tp,
                    func=AF.Copy)

            def tail_wo(qb):
                ic = NQB + qb
                for nh in range(2):
                    u = qb * 2 + nh
                    ps = tail_psum([128, 512], f32)
                    for g in range(2):
                        nc.tensor.matmul(
                            ps,
                            lhsT=ctx_t[1][:, g, qb * 128:(qb + 1) * 128],
                            rhs=wo_sb[:, g, nh * 512:(nh + 1) * 512],
                            start=(g == 0),
                            stop=(g == 1),
                        )
                    st_ = ostp.tile([128, 512], f16, tag="ost", name="st")
                    nc.vector.tensor_copy(out=st_, in_=ps)
                    nc.sync.dma_start(
                        out=out_d[ic * 128:(ic + 1) * 128,
                                  nh * 512:(nh + 1) * 512],
                        in_=st_)

            # lag-1: each transpose is emitted before the previous block's
            # wo unit so the in-order PE queue never parks a wo matmul
            # (waiting on its ctx_t copy) in front of the next transpose
            tail_tp(0)
            for qb in range(NQB):
                if qb + 1 < NQB:
                    tail_tp(qb + 1)
                tail_wo(qb)
            while fillers:
                pull(1)

    nc.compile()
    return nc


def _get_nc(debug=False):
    key = ("nc", debug)
    if key not in _cached:
        _cached[key] = _build(debug)
    return _cached[key]


def _get_runner():
    """Build (once) a jitted 8-core SPMD executable mirroring
    bass2jax.run_bass_via_pjrt, reusable across calls for benchmarking."""
    if "runner" in _cached:
        return _cached["runner"]
    import jax
    import jax.numpy as jnp
    from jax.experimental.shard_map import shard_map
    from jax.sharding import Mesh, PartitionSpec
    import concourse.mybir as mybir
    from concourse import bass2jax

    bass2jax.install_neuronx_cc_hook()
    nc = _get_nc()
    assert nc.dbg_addr is None
    partition_name = nc.partition_id_tensor.name if nc.partition_id_tensor else None

    in_names, out_names, out_avals, zero_outs = [], [], [], []
    for alloc in nc.m.functions[0].allocations:
        if not isinstance(alloc, mybir.MemoryLocationSet):
            continue
        name = alloc.memorylocations[0].name
        if alloc.kind == "ExternalInput":
            if name != partition_name:
                in_names.append(name)
        elif alloc.kind == "ExternalOutput":
            out_names.append(name)
            shape = tuple(alloc.tensor_shape)
            dtype = mybir.dt.np(alloc.dtype)
            out_avals.append(jax.core.ShapedArray(shape, dtype))
            zero_outs.append(np.zeros(shape, dtype))
    n_params = len(in_names)
    all_in_names = in_names + out_names
    if partition_name is not None:
        all_in_names = all_in_names + [partition_name]
    donate = tuple(range(n_params, n_params + len(out_names)))

    def _body(*args):
        operands = list(args)
        if partition_name is not None:
            operands.append(bass2jax.partition_id_tensor())
        outs = bass2jax._bass_exec_p.bind(
            *operands,
            out_avals=tuple(out_avals),
            in_names=tuple(all_in_names),
            out_names=tuple(out_names),
            lowering_input_output_aliases=(),
            sim_require_finite=True,
            sim_require_nnan=True,
            nc=nc,
        )
        return tuple(outs)

    devices = jax.devices()[:NC]
    mesh = Mesh(np.asarray(devices), ("core",))
    nin = n_params + len(out_names)
    sharded = jax.jit(
        shard_map(
            _body,
            mesh=mesh,
            in_specs=(PartitionSpec("core"),) * nin,
            out_specs=(PartitionSpec("core"),) * len(out_names),
            check_rep=False,
        ),
        donate_argnums=donate,
        keep_unused=True,
    )

    def run(in_maps):
        concat_in = [
            np.concatenate([np.asarray(in_maps[c][n]) for c in range(NC)], axis=0)
            for n in in_names
        ]
        concat_zeros = [
            np.zeros((NC * z.shape[0], *z.shape[1:]), z.dtype) for z in zero_outs
        ]
        out_arrs = sharded(*concat_in, *concat_zeros)
        return [
            {
                n: np.asarray(out_arrs[i]).reshape(NC, *out_avals[i].shape)[c]
                for i, n in enumerate(out_names)
            }
            for c in range(NC)
        ]

    _cached["runner"] = (run, sharded, in_names, out_names, out_avals, zero_outs)
    return _cached["runner"]


def _make_in_maps(query, key, value, Wq, bq, Wk, bk, Wv, bv, Wo, bo):
    import ml_dtypes

    query = np.asarray(query, dtype=np.float32)
    key = np.asarray(key, dtype=np.float32)
    value = np.asarray(value, dtype=np.float32)
    Wq, Wk, Wv, Wo = (np.asarray(a, dtype=np.float32) for a in (Wq, Wk, Wv, Wo))
    bq, bk, bv, bo = (np.asarray(a, dtype=np.float32) for a in (bq, bk, bv, bo))
    B = query.shape[0]
    ident = np.eye(128, dtype=ml_dtypes.bfloat16)
    xdt = ml_dtypes.bfloat16 if IN_BF16 else np.float32

    xqT = [np.ascontiguousarray(query[b].T).astype(xdt) for b in range(B)]
    xkT = [np.ascontiguousarray(key[b].T).astype(xdt) for b in range(B)]
    xvT = [np.ascontiguousarray(value[b].T).astype(ml_dtypes.bfloat16)
           for b in range(B)]

    in_maps = []
    for c in range(NC):
        b, hg = divmod(c, NC // B)
        sl = slice(hg * M, (hg + 1) * M)
        in_maps.append(
            {
                "xqT": xqT[b],
                "xkT": xkT[b],
                "xvT": xvT[b],
                "wq": np.ascontiguousarray(Wq[:, sl]).astype(xdt),
                "wk": np.ascontiguousarray(Wk[:, sl]).astype(xdt),
                "wv": np.ascontiguousarray(Wv[:, sl]).astype(ml_dtypes.bfloat16),
                "wo": np.ascontiguousarray(Wo[sl, :]).astype(ml_dtypes.bfloat16),
                "bq": np.ascontiguousarray(bq[sl]),
                "bk": np.ascontiguousarray(bk[sl]),
                "bvb": np.tile(bv[sl][None, :], (128, 1)),
                "ident": ident,
            }
        )
    return in_maps


def kernel(query, key, value, Wq, bq, Wk, bk, Wv, bv, Wo, bo):
    in_maps = _make_in_maps(query, key, value, Wq, bq, Wk, bk, Wv, bv, Wo, bo)
    run = _get_runner()[0]
    results = run(in_maps)

    B = np.asarray(query).shape[0]
    bo = np.asarray(bo, dtype=np.float32)
    full = np.zeros((B, S, D), np.float32)
    for b in range(B):
        acc = np.zeros((S, D), np.float32)
        for g in range(NC // B):
            acc += results[b * (NC // B) + g]["out"]
        full[b] = acc + bo[None, :]
    return full


# revision 65
# speedup vs baseline: 1.0062x; 1.0062x over previous
"""Multi-head attention (B=2, S=2048, D=1024, H=16) on 8 TRN2 NeuronCores.

Sharding: (batch, head-group) - core c handles batch c//4 and heads
[4*(c%4), 4*(c%4)+4). Each core projects its batch's tokens onto its 4 heads'
column-shards of Wq/Wk/Wv, runs attention for those heads, and multiplies by
its row-shard of Wo, producing a partial [S, D] output. The host sums the 4
partials per batch and adds bo. No FLOP duplication across cores.

Device design notes (v2, e-stationary PV):
  - Q/K are projected feature-major (qT/kT [dims, tokens] f32) so QK^T streams
    queries: scores^T [keys, queries] per 128-key block, exp'd on ACT into
    bf16 e tiles [128 keys, 1024 queries].
  - PV uses e as the STATIONARY operand: ctx[q, d] = e_blk^T @ v_blk with
    v [128 keys, 64 dims] as the moving operand (N=64), accumulated over key
    blocks in PSUM. Output lands queries-on-partitions, so the softmax
    denominator divide is a per-partition tensor_scalar multiply (no
    partition broadcasts). Denominators come from parallel N=1 matmuls
    (e_blk^T @ ones) accumulated in a dedicated PSUM bank.
  - V is projected token-major (x-chunk stationary, Wv moving, N=256), which
    directly yields v [tokens, dims] - no V transposes.
  - Normalized ctx pairs are PE-transposed ([q, dims] -> [dims, q]) into the
    packed ctx_t layout for the row-sharded Wo matmul (bf16).
  - The j-loop is split in two rounds (key halves) so attention overlaps the
    input-DMA ramp; round-1 ctx partials are evicted to SBUF and re-added
    during round 2. Denominators accumulate across both rounds in PSUM.
  - PSUM budget (8 banks): qk 2x[128,1024] (4) + ctx 2x[128,512] (2) +
    denominators (1) + scratch for proj/wo/transpose groups (1).  The ramp
    projections trickle per-DMA-chunk into the (still unused) qk psum slots;
    late projections run group-at-a-time from persistent stage tiles through
    the scratch bank so no psum slot is ever held across interleaved work.
  - Eviction work is spread: ACT (ramp proj bias adds), DVE (late proj bias,
    V bias adds, R1 evict, R2 add, reciprocal, ctx_t + Wo psum evictions),
    Pool/gpsimd (normalize multiplies - SBUF-only, since gpsimd has no PSUM
    port).
"""

import numpy as np

S = 2048          # sequence length
D = 1024          # model dim
HPC = 4           # heads per core
DK = 64           # head dim
M = HPC * DK      # per-core projection width = 256
NC = 8            # cores
IH = S // 2       # query half width (free dim of qk/exp tiles)
NQB = IH // 128   # 8 query blocks per half
NDC = D // 128    # 8 contraction chunks

IN_BF16 = True    # stream q/k/v inputs (and Wq/Wk) as bf16

_cached = {}


def _build(debug=False):
    import concourse.bass as bass
    import concourse.bacc as bacc
    import concourse.tile as tile
    import concourse.mybir as mybir
    from contextlib import ExitStack
    from collections import deque

    f32 = mybir.dt.float32
    f32r = mybir.dt.float32r
    bf16 = mybir.dt.bfloat16
    f16 = mybir.dt.float16
    AF = mybir.ActivationFunctionType

    xdt = bf16 if IN_BF16 else f32

    def r(ap):
        # moving/stationary f32 operands go through the PE at full rate as f32r
        return ap.bitcast(f32r) if ap.dtype == f32 else ap

    nc = bacc.Bacc(
        "TRN2",
        target_bir_lowering=False,
        debug=False,
        enable_asserts=False,
        num_devices=NC,
    )

    xqT_d = nc.dram_tensor("xqT", [D, S], xdt, kind="ExternalInput").ap()
    xkT_d = nc.dram_tensor("xkT", [D, S], xdt, kind="ExternalInput").ap()
    xvT_d = nc.dram_tensor("xvT", [D, S], bf16, kind="ExternalInput").ap()
    wq_d = nc.dram_tensor("wq", [D, M], xdt, kind="ExternalInput").ap()
    wk_d = nc.dram_tensor("wk", [D, M], xdt, kind="ExternalInput").ap()
    wv_d = nc.dram_tensor("wv", [D, M], bf16, kind="ExternalInput").ap()
    wo_d = nc.dram_tensor("wo", [M, D], bf16, kind="ExternalInput").ap()
    bq_d = nc.dram_tensor("bq", [M], f32, kind="ExternalInput").ap()
    bk_d = nc.dram_tensor("bk", [M], f32, kind="ExternalInput").ap()
    bvb_d = nc.dram_tensor("bvb", [128, M], f32, kind="ExternalInput").ap()
    ident_d = nc.dram_tensor("ident", [128, 128], bf16, kind="ExternalInput").ap()
    out_d = nc.dram_tensor("out", [S, D], f16, kind="ExternalOutput").ap()

    with tile.TileContext(nc) as tc:
        with ExitStack() as st:
            # ---- SBUF pools ----
            pw = st.enter_context(tc.tile_pool(name="pw", bufs=1))
            pqk = st.enter_context(tc.tile_pool(name="pqk", bufs=1))
            pvs = st.enter_context(tc.tile_pool(name="pvs", bufs=1))
            pxv = st.enter_context(tc.tile_pool(name="pxv", bufs=1))
            pstg = st.enter_context(tc.tile_pool(name="pstg", bufs=1))
            pct = st.enter_context(tc.tile_pool(name="pct", bufs=1))
            xt = st.enter_context(tc.tile_pool(name="xt", bufs=8))
            ep = st.enter_context(tc.tile_pool(name="ep", bufs=18))
            cpp = st.enter_context(tc.tile_pool(name="cpp", bufs=5))
            tmpp = st.enter_context(tc.tile_pool(name="tmpp", bufs=2))
            invp = st.enter_context(tc.tile_pool(name="invp", bufs=2))
            ostp = st.enter_context(tc.tile_pool(name="ostp", bufs=6))
            # ---- PSUM pools (8 banks total) ----
            qp = st.enter_context(tc.tile_pool(name="qp", bufs=2, space="PSUM"))
            cxp = st.enter_context(tc.tile_pool(name="cxp", bufs=2, space="PSUM"))
            dnp = st.enter_context(tc.tile_pool(name="dnp", bufs=1, space="PSUM"))
            pps = st.enter_context(tc.tile_pool(name="pps", bufs=1, space="PSUM"))

            # ---- persistent SBUF tiles ----
            qT = [[pqk.tile([128, IH], f32r, tag=f"qT{m}{s}", name=f"qT{m}{s}")
                   for s in range(2)] for m in range(2)]
            kT = [[pqk.tile([128, IH], f32r, tag=f"kT{m}{s}", name=f"kT{m}{s}")
                   for s in range(2)] for m in range(2)]
            v_sb = [[pvs.tile([128, 8, DK], bf16, tag=f"v{h}{s}", name=f"v{h}{s}")
                     for s in range(2)] for h in range(HPC)]
            ctx_t = [pct.tile([128, 2, IH], bf16, tag=f"ctxt{i}", name=f"ctxt{i}")
                     for i in range(2)]
            cpair = [[pct.tile([128, NQB, 128], bf16, tag=f"cp{i}{m}",
                               name=f"cp{i}{m}") for m in range(2)]
                     for i in range(2)]

            wq_sb = pw.tile([128, NDC, M], xdt, tag="wq")
            wk_sb = pw.tile([128, NDC, M], xdt, tag="wk")
            wv_sb = pw.tile([128, NDC, M], bf16, tag="wv")
            wo_sb = pw.tile([128, 2, D], bf16, tag="wo")
            bq_sb = pw.tile([128, 2], f32, tag="bq")
            bk_sb = pw.tile([128, 2], f32, tag="bk")
            bvb_sb = pw.tile([128, M], f32, tag="bvb")
            ident = pw.tile([128, 128], bf16, tag="ident")
            ones = pw.tile([128, 1], bf16, tag="ones")

            # denominator accumulator: col = ih*32 + h*8 + qb
            dn = dnp.tile([128, 64], f32, tag="dn", name="dn")

            w_r = lambda ap: ap.rearrange("(n p) m -> p n m", p=128)

            nc.vector.memset(ones, 1.0)

            # ---------------- emission helpers ----------------
            fillers = deque()

            def pull(n=1):
                for _ in range(n):
                    while fillers:
                        try:
                            next(fillers[0])
                            break
                        except StopIteration:
                            fillers.popleft()
                    else:
                        return

            qchunks = []
            kchunks = []

            def ramp_qk_proj(tens, mcs):
                """Ramp projection of q/k token-half 0: x chunks trickle from
                DMA straight into accumulating matmuls hosted in the (still
                free) qk psum slots.  Runs before any attention emission.
                Only head-pairs in `mcs` are projected; for q, mc1 is
                deferred to a filler (the first attention heads are mc0)."""
                xdram = xqT_d if tens == "q" else xkT_d
                w_sb = wq_sb if tens == "q" else wk_sb
                b_sb = bq_sb if tens == "q" else bk_sb
                dst = qT if tens == "q" else kT
                ps = {mc: qp.tile([128, IH], f32, tag="qk", name=f"pj{tens}{mc}")
                      for mc in mcs}
                for dc in range(NDC):
                    xc = xt.tile([128, IH], xdt, tag="x", name="x")
                    nc.sync.dma_start(out=xc, in_=xdram[dc * 128:(dc + 1) * 128, 0:IH])
                    (qchunks if tens == "q" else kchunks).append(xc)
                    for mc in mcs:
                        for sc in range(2):
                            nc.tensor.matmul(
                                ps[mc][:, sc * 512:(sc + 1) * 512],
                                lhsT=r(w_sb[:, dc, mc * 128:(mc + 1) * 128]),
                                rhs=r(xc[:, sc * 512:(sc + 1) * 512]),
                                start=(dc == 0),
                                stop=(dc == NDC - 1),
                            )
                for mc in mcs:
                    # sc0 on ACT / sc1 on DVE: the two halves evict in
                    # parallel so first-exp follows the last matmul quickly
                    nc.scalar.add(
                        out=dst[mc][0][:, 0:512],
                        in_=ps[mc][:, 0:512],
                        add=b_sb[:, mc:mc + 1])
                    nc.vector.tensor_scalar_add(
                        out=dst[mc][0][:, 512:1024],
                        in0=ps[mc][:, 512:1024],
                        scalar1=b_sb[:, mc:mc + 1])

            def emit_late_mc1(tens):
                """Deferred mc1 projection of q/k half-0 from the saved ramp
                chunks, one group at a time through the scratch bank."""
                w_sb = wq_sb if tens == "q" else wk_sb
                b_sb = bq_sb if tens == "q" else bk_sb
                dst = (qT if tens == "q" else kT)[1][0]
                chunks = qchunks if tens == "q" else kchunks
                for sc in range(2):
                    ps = pps.tile([128, 512], f32, tag="ps", name=f"{tens}mc1")
                    for dc in range(NDC):
                        nc.tensor.matmul(
                            ps,
                            lhsT=r(w_sb[:, dc, 128:256]),
                            rhs=r(chunks[dc][:, sc * 512:(sc + 1) * 512]),
                            start=(dc == 0),
                            stop=(dc == NDC - 1),
                        )
                        if dc == 3:
                            yield
                    nc.vector.tensor_scalar_add(
                        out=dst[:, sc * 512:(sc + 1) * 512],
                        in0=ps, scalar1=b_sb[:, 1:2])
                    yield

            stg_tiles = {}

            def emit_stage_dma(tens):
                """DMA the token-half-1 x chunks of q/k into a persistent
                stage tile (SP queue only - no engine work)."""
                xdram = xqT_d if tens == "q" else xkT_d
                stg = pstg.tile([128, NDC, IH], xdt, tag="stg", name=f"stg{tens}")
                for dc in range(NDC):
                    nc.sync.dma_start(
                        out=stg[:, dc, :],
                        in_=xdram[dc * 128:(dc + 1) * 128, IH:S])
                    yield
                stg_tiles[tens] = stg

            def emit_late_proj(tens):
                """Token-half-1 projection of q/k from the stage tile,
                one (mc, sc) accumulation group at a time through the
                scratch psum bank."""
                w_sb = wq_sb if tens == "q" else wk_sb
                b_sb = bq_sb if tens == "q" else bk_sb
                dst = qT if tens == "q" else kT
                stg = stg_tiles[tens]
                for mc in range(2):
                    for sc in range(2):
                        ps = pps.tile([128, 512], f32, tag="ps", name=f"lp{tens}")
                        for dc in range(NDC):
                            nc.tensor.matmul(
                                ps,
                                lhsT=r(w_sb[:, dc, mc * 128:(mc + 1) * 128]),
                                rhs=r(stg[:, dc, sc * 512:(sc + 1) * 512]),
                                start=(dc == 0),
                                stop=(dc == NDC - 1),
                            )
                            if dc % 2 == 1:
                                yield
                        nc.vector.tensor_scalar_add(
                            out=dst[mc][1][:, sc * 512:(sc + 1) * 512],
                            in0=ps, scalar1=b_sb[:, mc:mc + 1])
                        yield

            xv_tiles = {}

            def emit_xv_dma(sh):
                xv = pxv.tile([128, NDC, IH], bf16, tag="xv", name=f"xv{sh}")
                for dc in range(NDC):
                    nc.sync.dma_start(
                        out=xv[:, dc, :],
                        in_=xvT_d[dc * 128:(dc + 1) * 128, sh * IH:(sh + 1) * IH])
                    yield
                xv_tiles[sh] = xv

            def emit_v_proj(sh):
                """Token-major V projection: two token-blocks per pps tile."""
                xv = xv_tiles[sh]
                for tbp in range(4):
                    ps = pps.tile([128, 512], f32, tag="ps", name="vps")
                    for dc in range(NDC):
                        for j in range(2):
                            tb = tbp * 2 + j
                            # the two token-blocks share one psum bank:
                            # single start (j0/dc0) and stop (j1/dc7)
                            nc.tensor.matmul(
                                ps[:, j * M:(j + 1) * M],
                                lhsT=xv[:, dc, tb * 128:(tb + 1) * 128],
                                rhs=wv_sb[:, dc, :],
                                start=(dc == 0 and j == 0),
                                stop=(dc == NDC - 1 and j == 1),
                            )
                        if dc % 2 == 1:
                            yield
                    for j in range(2):
                        tb = tbp * 2 + j
                        for h in range(HPC):
                            nc.vector.tensor_add(
                                v_sb[h][sh][:, tb, :],
                                ps[:, j * M + h * DK:j * M + (h + 1) * DK],
                                bvb_sb[:, h * DK:(h + 1) * DK])
                    yield

            def emit_wo_dma():
                nc.sync.dma_start(out=wo_sb, in_=w_r(wo_d))
                yield

            def emit_wo(ih, icb, tail=False):
                """One token-block of the output projection (both D halves).

                In tail mode the two psum groups alternate between a qk slot
                (free by then) and the scratch bank, and the evictions
                alternate ACT/DVE, so the final token-blocks pipeline instead
                of serializing on one bank + one engine."""
                ic = ih * NQB + icb
                for nh in range(2):
                    if tail and nh == 0:
                        ps = qp.tile([128, 512], f32, tag="qk", name="wops")
                    else:
                        ps = pps.tile([128, 512], f32, tag="ps", name="wops")
                    for g in range(2):
                        nc.tensor.matmul(
                            ps,
                            lhsT=ctx_t[ih][:, g, icb * 128:(icb + 1) * 128],
                            rhs=wo_sb[:, g, nh * 512:(nh + 1) * 512],
                            start=(g == 0),
                            stop=(g == 1),
                        )
                    st_ = ostp.tile([128, 512], f16, tag="ost", name="st")
                    if tail and nh == 1:
                        nc.scalar.activation(out=st_, in_=ps, func=AF.Copy)
                    else:
                        nc.vector.tensor_copy(out=st_, in_=ps)
                    # out-DMAs ride the SP queue: issuing from the ACT queue
                    # would stall the exp decode stream ~650ns per DMA
                    nc.sync.dma_start(
                        out=out_d[ic * 128:(ic + 1) * 128, nh * 512:(nh + 1) * 512],
                        in_=st_)
                    yield

            cxs = {}      # (ih, h) -> live ctx psum tile
            cps = {}      # (ih, h) -> R1 partial in SBUF

            def emit_qk_exp(ih, h, kb):
                sh, kbl = divmod(kb, 8)
                mc, off = divmod(h, 2)
                off *= DK
                qk = qp.tile([128, IH], f32, tag="qk", name="qk")
                for ha in range(2):
                    nc.tensor.matmul(
                        qk[:, ha * 512:(ha + 1) * 512],
                        lhsT=r(kT[mc][sh][off:off + DK, kbl * 128:(kbl + 1) * 128]),
                        rhs=r(qT[mc][ih][off:off + DK, ha * 512:(ha + 1) * 512]),
                        start=True, stop=True,
                    )
                e = ep.tile([128, IH], bf16, tag="e", name="e")
                nc.scalar.activation(out=e, in_=qk, func=AF.Exp, scale=1.0 / 8.0)
                return e

            dn_state = {"started": False}

            def emit_pv(ih, h, kb, e):
                # PSUM group semantics: a start=True matmul zeroes the whole
                # 2KB bank, so exactly ONE start (first matmul into the bank)
                # and ONE stop (last matmul) per bank-lifetime; all sibling
                # accumulators in the bank ride the same group and simply
                # accumulate onto the zeroed region.
                sh = kb // 8
                if kb % 8 == 0:
                    cxs[(ih, h)] = cxp.tile([128, 512], f32, tag="ctx",
                                            name=f"cx{ih}{h}{kb}")
                cx = cxs[(ih, h)]
                dbase = ih * 32 + h * 8
                for qb in range(NQB):
                    lhs = e[:, qb * 128:(qb + 1) * 128]
                    nc.tensor.matmul(
                        cx[:, qb * DK:(qb + 1) * DK],
                        lhsT=lhs,
                        rhs=v_sb[h][sh][:, kb % 8, :],
                        start=(kb % 8 == 0 and qb == 0),
                        stop=(kb % 8 == 7 and qb == NQB - 1),
                    )
                    dn_start = not dn_state["started"]
                    dn_state["started"] = True
                    # the bank-wide group stays open across the whole kernel
                    # (per-slot reads are ordered by data deps, so skip the
                    # interp's group-state checks for these)
                    nc.tensor.matmul(
                        dn[:, dbase + qb:dbase + qb + 1],
                        lhsT=lhs,
                        rhs=ones,
                        start=dn_start,
                        stop=(ih == 1 and h == 3 and kb == 15 and qb == NQB - 1),
                        skip_group_check=True,
                    )

            def emit_evict_r1(ih, h):
                cx = cxs.pop((ih, h))
                cp = cpp.tile([128, 512], f32, tag="cp", name=f"cp{ih}{h}")
                nc.vector.tensor_copy(out=cp, in_=cx)
                cps[(ih, h)] = cp

            def emit_norm(ih, h):
                cx = cxs.pop((ih, h))
                cp = cps.pop((ih, h))
                mc, off = divmod(h, 2)
                off *= DK
                dbase = ih * 32 + h * 8
                inv = invp.tile([128, NQB], f32, tag="inv", name="inv")
                nc.vector.reciprocal(out=inv, in_=dn[:, dbase:dbase + NQB])
                tm = tmpp.tile([128, 512], f32, tag="tmp", name="tm")
                nc.vector.tensor_add(tm, cx, cp)
                for qb in range(NQB):
                    nc.gpsimd.tensor_scalar_mul(
                        out=cpair[ih][mc][:, qb, off:off + DK],
                        in0=tm[:, qb * DK:(qb + 1) * DK],
                        scalar1=inv[:, qb:qb + 1])

            def emit_tp(ih, mc, qb):
                # transposes borrow a ctx psum slot (never the scratch bank,
                # which may be mid-accumulation inside a filler generator)
                tp = cxp.tile([128, 128], bf16, tag="ctx", name="tp")
                nc.tensor.transpose(tp, in_=cpair[ih][mc][:, qb, :], identity=ident)
                nc.vector.tensor_copy(
                    out=ctx_t[ih][:, mc, qb * 128:(qb + 1) * 128], in_=tp)

            def emit_tp_half(ih, mc, qb, half, eng="dve"):
                # one head's 64-dim half of a pair transpose: [128 q, 64 d]
                # -> psum partitions half*64..half*64+64, then a partition-
                # sliced copy into ctx_t.  Lets the tail pair's even head
                # transpose early, leaving only the odd half on the tail.
                tp = cxp.tile([128, 128], bf16, tag="ctx", name="tph")
                nc.tensor.transpose(
                    tp[half * DK:(half + 1) * DK, :],
                    in_=cpair[ih][mc][:, qb, half * DK:(half + 1) * DK],
                    identity=ident)
                dst = ctx_t[ih][half * DK:(half + 1) * DK, mc,
                               qb * 128:(qb + 1) * 128]
                if eng == "act":
                    nc.scalar.activation(
                        out=dst, in_=tp[half * DK:(half + 1) * DK, :],
                        func=AF.Copy)
                else:
                    nc.vector.tensor_copy(
                        out=dst, in_=tp[half * DK:(half + 1) * DK, :])

            # ---------------- global schedule ----------------
            # PE p-state: the cost model resets the tensor clock to 0.65/1.2
            # GHz after any dependency wait and needs ~3us of continuous
            # execution to reach 2.4 GHz.  An initial burst of junk matmuls
            # on a zeroed tile warms the engine before the chunk-paced
            # projections; from then on 4 matmuls per 728ns chunk period
            # keep the engine saturated (never waiting, never resetting).
            junk = pw.tile([128, 512], bf16, tag="junk")
            nc.vector.memset(junk, 0.0)
            jps = pps.tile([128, 512], f32, tag="ps", name="jps")
            for _ in range(8):
                nc.tensor.matmul(jps, lhsT=junk[:, 0:128], rhs=junk,
                                 start=True, stop=True)
            nc.sync.dma_start(out=wk_sb, in_=w_r(wk_d))
            nc.sync.dma_start(out=bk_sb, in_=bk_d.rearrange("(n p) -> p n", p=128))
            ramp_qk_proj("k", (0, 1))
            nc.sync.dma_start(out=wq_sb, in_=w_r(wq_d))
            nc.sync.dma_start(out=bq_sb, in_=bq_d.rearrange("(n p) -> p n", p=128))
            ramp_qk_proj("q", (0, 1))
            nc.sync.dma_start(out=wv_sb, in_=w_r(wv_d))
            nc.sync.dma_start(out=bvb_sb, in_=bvb_d)
            nc.sync.dma_start(out=ident, in_=ident_d)
            for _ in emit_xv_dma(0):
                pass

            # fillers consumed inside attention (FIFO order matters: each
            # generator's data deps are satisfied by the time it is pulled)
            fillers.append(emit_stage_dma("k"))
            fillers.append(emit_v_proj(0))
            fillers.append(emit_late_proj("k"))
            fillers.append(emit_xv_dma(1))
            fillers.append(emit_v_proj(1))
            fillers.append(emit_wo_dma())
            fillers.append(emit_stage_dma("q"))
            fillers.append(emit_late_proj("q"))

            es = {}
            # ---- R1(ih0): heads 0,1 qk/exp only (V still streaming);
            # aggressive pulls here drain the deferred mc1 projections
            # before heads 2,3 need them ----
            for h in (0, 1):
                for kb in range(8):
                    es[(h, kb)] = emit_qk_exp(0, h, kb)
                    pull(3)
            # ---- heads 2,3 qk/exp, flushing heads 0,1 PV with a lag ----
            for h in (2, 3):
                for kb in range(8):
                    es[(h, kb)] = emit_qk_exp(0, h, kb)
                    emit_pv(0, h - 2, kb, es.pop((h - 2, kb)))
                    pull(2)
                emit_evict_r1(0, h - 2)

            # ---- Lag-2 software pipeline for the remaining three rounds:
            # the qk/exp of iterations i+1 AND i+2 are emitted before the pv
            # of iteration i, so each qk completes well inside the previous
            # exp's window and ACT never waits on the in-order PE queue.
            # drain(it) emits the pv (+ any round-boundary work) of `it`.
            def drain(it):
                ih, ph, pkb, pe = it
                extra = False
                if ih == 0 and pkb >= 8 and ph < 2:
                    # leftover R1 pv of heads 2,3 rides on heads 0,1 of R2
                    emit_pv(0, ph + 2, pkb - 8, es.pop((ph + 2, pkb - 8)))
                    extra = True
                    if pkb == 15:
                        emit_evict_r1(0, ph + 2)
                emit_pv(ih, ph, pkb, pe)
                if pkb == 7 and not (ih == 0 and ph >= 2):
                    emit_evict_r1(ih, ph)
                boundary = False
                if pkb == 15 and not (ih == 1 and ph == 3):
                    emit_norm(ih, ph)
                    if ph % 2 == 1:
                        for qb in range(NQB):
                            emit_tp(ih, ph // 2, qb)
                        boundary = True
                if not boundary:
                    pull(2 if (ih == 0 and ph == 0) else 1)

            seq = ([(0, h, kb) for h in range(HPC) for kb in range(8, 16)]
                   + [(1, h, kb) for h in range(HPC) for kb in range(8)]
                   + [(1, h, kb) for h in range(HPC) for kb in range(8, 16)])
            wo0 = False
            pend = deque()
            for ih, h, kb in seq:
                if not wo0 and (ih, h, kb) == (1, 0, 2):
                    for icb in range(NQB):
                        fillers.append(emit_wo(0, icb))
                    wo0 = True
                e = emit_qk_exp(ih, h, kb)
                pend.append((ih, h, kb, e))
                if len(pend) > 2:
                    drain(pend.popleft())
            while pend:
                drain(pend.popleft())

            # ---- tail: normalize the last head per query block and
            # immediately transpose + project + store that block.  Everything
            # is per-qb so the 6-stage chain (DVE add -> Pool mul -> PE
            # transpose -> DVE copy -> PE wo -> ACT/DVE evict -> DMA)
            # pipelines across engines; wo psums rotate over the qk/ctx/
            # scratch banks (all free by now) and evictions alternate
            # ACT/DVE so no single bank or engine serializes the tail. ----
            inv = invp.tile([128, NQB], f32, tag="inv", name="inv")
            nc.vector.reciprocal(out=inv, in_=dn[:, 56:64])
            cx = cxs.pop((1, 3))
            cp = cps.pop((1, 3))
            # Pool pre-scales the R1 partial by 1/denom so one fused DVE
            # scalar_tensor_tensor per block does (psum*inv + partial*inv)
            tm = tmpp.tile([128, 512], f32, tag="tmp", name="tm")

            def tail_psum(u):
                if u % 3 == 0:
                    return qp.tile([128, 512], f32, tag="qk", name="wops")
                if u % 3 == 1:
                    return cxp.tile([128, 512], f32, tag="ctx", name="wops")
                return pps.tile([128, 512], f32, tag="ps", name="wops")

            mult_op = mybir.AluOpType.mult
            add_op = mybir.AluOpType.add
            # phase A: the whole normalize as an uninterrupted DVE/Pool burst
            # (no wo-evictions queued behind it to couple the stages)
            for qb in range(NQB):
                nc.gpsimd.tensor_scalar_mul(
                    out=tm[:, qb * DK:(qb + 1) * DK],
                    in0=cp[:, qb * DK:(qb + 1) * DK],
                    scalar1=inv[:, qb:qb + 1])
                nc.vector.scalar_tensor_tensor(
                    out=cpair[1][1][:, qb, DK:2 * DK],
                    in0=cx[:, qb * DK:(qb + 1) * DK],
                    scalar=inv[:, qb:qb + 1],
                    in1=tm[:, qb * DK:(qb + 1) * DK],
                    op0=mult_op, op1=add_op)
            # phase B: per-qb transpose + wo, psums rotating over all five
            # free psum slots, evictions alternating ACT/DVE
            nslot = 0

            def tail_psum(shape, dtype):
                nonlocal nslot
                nslot += 1
                if nslot % 5 in (0, 1):
                    return qp.tile(shape, dtype, tag="qk", name="wops")
                if nslot % 5 in (2, 3):
                    return cxp.tile(shape, dtype, tag="ctx", name="wops")
                return pps.tile(shape, dtype, tag="ps", name="wops")

            def tail_tp(qb):
                tp = tail_psum([128, 128], bf16)
                nc.tensor.transpose(tp, in_=cpair[1][1][:, qb, :], identity=ident)
                if qb % 2 == 0:
                    nc.scalar.activation(
                        out=ctx_t[1][:, 1, qb * 128:(qb + 1) * 128], in_=tp,
                        func=AF.Copy)
                else:
                    nc.vector.tensor_copy(
                        out=ctx_t[1][:, 1, qb * 128:(qb + 1) * 128], in_=tp)

            def tail_wo(qb):
                ic = NQB + qb
                for nh in range(2):
                    u = qb * 2 + nh
                    ps = tail_psum([128, 512], f32)
                    for g in range(2):
                        nc.tensor.matmul(
                            ps,
                            lhsT=ctx_t[1][:, g, qb * 128:(qb + 1) * 128],
                            rhs=wo_sb[:, g, nh * 512:(nh + 1) * 512],
                            start=(g == 0),
                            stop=(g == 1),
                        )
                    st_ = ostp.tile([128, 512], f16, tag="ost", name="st")
                    if u % 2 == 0:
                        nc.vector.tensor_copy(out=st_, in_=ps)
                    else:
                        nc.scalar.activation(out=st_, in_=ps, func=AF.Copy)
                    nc.sync.dma_start(
                        out=out_d[ic * 128:(ic + 1) * 128,
                                  nh * 512:(nh + 1) * 512],
                        in_=st_)

            # lag-1: each transpose is emitted before the previous block's
            # wo unit so the in-order PE queue never parks a wo matmul
            # (waiting on its ctx_t copy) in front of the next transpose
            tail_tp(0)
            for qb in range(NQB):
                if qb + 1 < NQB:
                    tail_tp(qb + 1)
                tail_wo(qb)
            while fillers:
                pull(1)

    nc.compile()
    return nc


def _get_nc(debug=False):
    key = ("nc", debug)
    if key not in _cached:
        _cached[key] = _build(debug)
    return _cached[key]


def _get_runner():
    """Build (once) a jitted 8-core SPMD executable mirroring
    bass2jax.run_bass_via_pjrt, reusable across calls for benchmarking."""
    if "runner" in _cached:
        return _cached["runner"]
    import jax
    import jax.numpy as jnp
    from jax.experimental.shard_map import shard_map
    from jax.sharding import Mesh, PartitionSpec
    import concourse.mybir as mybir
    from concourse import bass2jax

    bass2jax.install_neuronx_cc_hook()
    nc = _get_nc()
    assert nc.dbg_addr is None
    partition_name = nc.partition_id_tensor.name if nc.partition_id_tensor else None

    in_names, out_names, out_avals, zero_outs = [], [], [], []
    for alloc in nc.m.functions[0].allocations:
        if not isinstance(alloc, mybir.MemoryLocationSet):
            continue
        name = alloc.memorylocations[0].name
        if alloc.kind == "ExternalInput":
            if name != partition_name:
                in_names.append(name)
        elif alloc.kind == "ExternalOutput":
            out_names.append(name)
            shape = tuple(alloc.tensor_shape)
            dtype = mybir.dt.np(alloc.dtype)
            out_avals.append(jax.core.ShapedArray(shape, dtype))
            zero_outs.append(np.zeros(shape, dtype))
    n_params = len(in_names)
    all_in_names = in_names + out_names
    if partition_name is not None:
        all_in_names = all_in_names + [partition_name]
    donate = tuple(range(n_params, n_params + len(out_names)))

    def _body(*args):
        operands = list(args)
        if partition_name is not None:
            operands.append(bass2jax.partition_id_tensor())
        outs = bass2jax._bass_exec_p.bind(
            *operands,
            out_avals=tuple(out_avals),
            in_names=tuple(all_in_names),
            out_names=tuple(out_names),
            lowering_input_output_aliases=(),
            sim_require_finite=True,
            sim_require_nnan=True,
            nc=nc,
        )
        return tuple(outs)

    devices = jax.devices()[:NC]
    mesh = Mesh(np.asarray(devices), ("core",))
    nin = n_params + len(out_names)
    sharded = jax.jit(
        shard_map(
            _body,
            mesh=mesh,
            in_specs=(PartitionSpec("core"),) * nin,
            out_specs=(PartitionSpec("core"),) * len(out_names),
            check_rep=False,
        ),
        donate_argnums=donate,
        keep_unused=True,
    )

    def run(in_maps):
        concat_in = [
            np.concatenate([np.asarray(in_maps[c][n]) for c in range(NC)], axis=0)
            for n in in_names
        ]
        concat_zeros = [
            np.zeros((NC * z.shape[0], *z.shape[1:]), z.dtype) for z in zero_outs
        ]
        out_arrs = sharded(*concat_in, *concat_zeros)
        return [
            {
                n: np.asarray(out_arrs[i]).reshape(NC, *out_avals[i].shape)[c]
                for i, n in enumerate(out_names)
            }
            for c in range(NC)
        ]

    _cached["runner"] = (run, sharded, in_names, out_names, out_avals, zero_outs)
    return _cached["runner"]


def _make_in_maps(query, key, value, Wq, bq, Wk, bk, Wv, bv, Wo, bo):
    import ml_dtypes

    query = np.asarray(query, dtype=np.float32)
    key = np.asarray(key, dtype=np.float32)
    value = np.asarray(value, dtype=np.float32)
    Wq, Wk, Wv, Wo = (np.asarray(a, dtype=np.float32) for a in (Wq, Wk, Wv, Wo))
    bq, bk, bv, bo = (np.asarray(a, dtype=np.float32) for a in (bq, bk, bv, bo))
    B = query.shape[0]
    ident = np.eye(128, dtype=ml_dtypes.bfloat16)
    xdt = ml_dtypes.bfloat16 if IN_BF16 else np.float32

    xqT = [np.ascontiguousarray(query[b].T).astype(xdt) for b in range(B)]
    xkT = [np.ascontiguousarray(key[b].T).astype(xdt) for b in range(B)]
    xvT = [np.ascontiguousarray(value[b].T).astype(ml_dtypes.bfloat16)
           for b in range(B)]

    in_maps = []
    for c in range(NC):
        b, hg = divmod(c, NC // B)
        sl = slice(hg * M, (hg + 1) * M)
        in_maps.append(
            {
                "xqT": xqT[b],
                "xkT": xkT[b],
                "xvT": xvT[b],
                "wq": np.ascontiguousarray(Wq[:, sl]).astype(xdt),
                "wk": np.ascontiguousarray(Wk[:, sl]).astype(xdt),
                "wv": np.ascontiguousarray(Wv[:, sl]).astype(ml_dtypes.bfloat16),
                "wo": np.ascontiguousarray(Wo[sl, :]).astype(ml_dtypes.bfloat16),
                "bq": np.ascontiguousarray(bq[sl]),
                "bk": np.ascontiguousarray(bk[sl]),
                "bvb": np.tile(bv[sl][None, :], (128, 1)),
                "ident": ident,
            }
        )
    return in_maps


def kernel(query, key, value, Wq, bq, Wk, bk, Wv, bv, Wo, bo):
    in_maps = _make_in_maps(query, key, value, Wq, bq, Wk, bk, Wv, bv, Wo, bo)
    run = _get_runner()[0]
    results = run(in_maps)

    B = np.asarray(query).shape[0]
    bo = np.asarray(bo, dtype=np.float32)
    full = np.zeros((B, S, D), np.float32)
    for b in range(B):
        acc = np.zeros((S, D), np.float32)
        for g in range(NC // B):
            acc += results[b * (NC // B) + g]["out"]
        full[b] = acc + bo[None, :]
    return full
